# revision 2
# baseline (speedup 1.0000x reference)
"""Trainium2 Bass kernel: GQA attention over packed ragged sequences (v2).

Sharding: tensor-parallel over heads across 8 NeuronCores. Core c owns
q-heads [4c, 4c+4) and kv-head c. wq/wk/wv are sharded by head rows,
wo by columns; each core computes a full [S, DIM] partial of the output
projection and the partials are summed on the host (row-parallel
all-reduce of wo).

v2 changes over the baseline:
 - Attention computes scores TRANSPOSED: S^T[k, q] tiles directly
   (lhsT = k-tile, rhs = q columns). The ACT exp then evacuates score
   psum straight into the SBUF strips that the PV matmuls consume --
   the per-tile PE transposes, DVE strip copies and DVE normalize of
   the old layout are gone (one cross-engine hop instead of five).
 - Softmax denominators come from tiny [128,1] ones-matmuls per
   (k-tile, q-tile) accumulated in psum; the reciprocal row is
   transposed on PE and partition-broadcast on the idle Pool (gpsimd)
   engine; normalization folds into the PV psum->attT evacuation
   multiply on DVE.
 - Score psum rotation deepened to five 512-wide slots (big3 freed by
   dropping probability transposes).
 - Last-tq RoPE writes go to separate qTtL/kTtL tiles so tile-granular
   deps can't stall the first sequences' attention on the deferred
   rope burst.
 - Startup: first weight/x group split [1,1,2,4] chunks with the ident
   load deferred behind the first pieces (first matmul ~2us earlier).
"""

import os
import numpy as np
import ml_dtypes

import concourse.bass as bass
import concourse.mybir as mybir
from concourse.tile import TileContext, add_dep_helper
from concourse.bass_utils import run_bass_kernel_spmd

F32 = mybir.dt.float32
BF16 = mybir.dt.bfloat16
AX = mybir.AxisListType
ALU = mybir.AluOpType
ACT = mybir.ActivationFunctionType

H, KVH, D, DIM = 32, 8, 128, 4096
NCORES = 8
HPC = H // NCORES          # q heads per core
S = 2048                   # total packed tokens
SCALE = D ** -0.5
P = 128                    # partition count / tile edge
KC = DIM // P              # contraction chunks for qkv projections
TQ, TW = 4, 512            # token quarters for projection phase
NG = 8                     # contraction chunks per DMA group
GPT = KC // NG             # groups per token quarter
WCOLS = HPC * D + 2 * D    # packed weight cols per chunk: q|k|v = 768
NEG = -1.0e30

LAST_RESULTS = None        # BassKernelResults of the most recent run


def _bfv(ap):
    """bf16 view of an f32 psum AP slice, full byte extent."""
    return ap.bitcast(BF16)


def _build(seq_tiles):
    """Build the per-core Bass program. seq_tiles: tiles (of 128 tokens)
    per packed sequence, e.g. (4, 6, 2, 4).

    Sync-wait discipline (walrus caps: PE matmul LW = 1 wait, DMA = 2):
    - every DMA-produced tile is first read by a tiny PE "touch" matmul
      into a persistent [1,1] psum scratch (absorbs the DMA wait);
    - all PSUM lives in persistent tiles from one global pool;
    - streamed SBUF tiles are persistent with manual rotation;
    - overflow waits are moved onto same-engine NoOps by _prune_waits.
    """
    LMAX = max(seq_tiles) * P
    assert LMAX <= 1024 and sum(seq_tiles) * P == S
    LQ0 = S - TW  # tokens >= LQ0 live in the deferred-rope L tiles
    # every sequence must fall wholly on one side of LQ0
    acc = 0
    for T in seq_tiles:
        assert acc + T * P <= LQ0 or acc >= LQ0, seq_tiles
        acc += T * P

    nc = bass.Bass()

    xTr = nc.dram_tensor("xTr", [KC, P, S], BF16, kind="ExternalInput")
    wcat = nc.dram_tensor("wcat", [KC, P, WCOLS], BF16, kind="ExternalInput")
    woTr = nc.dram_tensor("woTr", [HPC, P, DIM], BF16, kind="ExternalInput")
    cossin = nc.dram_tensor("cossin", [P, 2 * S], BF16, kind="ExternalInput")
    trim = nc.dram_tensor("trim", [P, P], BF16, kind="ExternalInput")
    identh = nc.dram_tensor("identh", [P, P], BF16, kind="ExternalInput")
    out_d = nc.dram_tensor("out", [S, DIM], BF16, kind="ExternalOutput")

    with TileContext(nc) as tc:
        with tc.tile_pool(name="glob", bufs=1) as gp, \
             tc.tile_pool(name="globps", space="PSUM", bufs=1) as gpp, \
             tc.tile_pool(name="qkv", bufs=1, side="right") as qkvp, \
             tc.tile_pool(name="woP", bufs=1) as wop:
            trimt = gp.tile([P, P], BF16, name="trimt")
            ident = gp.tile([P, P], BF16, name="ident")

            # q/k/v residents
            qTt = qkvp.tile([P, HPC * S], BF16, name="qTt")  # per-head [d, tok]
            kTt = qkvp.tile([P, S], BF16, name="kTt")        # [d, tok]
            qTtL = qkvp.tile([P, HPC * TW], BF16, name="qTtL")  # last-tq rope
            kTtL = qkvp.tile([P, TW], BF16, name="kTtL")
            vt = qkvp.tile([P, S], BF16, name="vt")          # [tok%128, blk*128+d]

            # full wo resident: head-chunk f at cols f*DIM + [0, DIM)
            woh = wop.tile([P, HPC * DIM], BF16, name="woh")

            # all PSUM, statically laid out
            big0 = gpp.tile([P, 1024], F32, name="big0")
            big1 = gpp.tile([P, 1024], F32, name="big1")
            big2 = gpp.tile([P, 1024], F32, name="big2")
            big3 = gpp.tile([P, 512], F32, name="big3")
            bigT = gpp.tile([P, 512], F32, name="bigT")
            # touch scratch kept clear of every live psum region
            tps = bigT[0:1, 384:512]

            def touch(t):
                # N=1 matmuls fail walrus's ISA check; use a [K,1]x[K,2] probe
                return nc.tensor.matmul(tps[0:1, 0:2], t[:, 0:1], t[:, 0:2],
                                        start=True, stop=True)

            # p-state warmup: the tensor engine's ramp clock starts at its
            # first instruction, so issue one trivial matmul immediately
            # (long before the first weights arrive) — the real first
            # matmuls then run at full rate instead of the 2-3.7x-slow
            # ramp states
            warm = gp.tile([P, 2], BF16, name="warm")
            wms = nc.vector.memset(warm[:, :], 0.0)
            wt = touch(warm)
            add_dep_helper(wt.ins, wms.ins, sync=True, reason="warm ready")

            # ---------------- Phase A: QKV projections + RoPE ----------
            with tc.tile_pool(name="wA", bufs=1) as wp, \
                 tc.tile_pool(name="xA", bufs=1) as xp, \
                 tc.tile_pool(name="csA", bufs=1) as csp, \
                 tc.tile_pool(name="ropeA", bufs=1) as rp:
                cst = gp.tile([P, 2 * S], BF16, name="cst")
                cos2t = cst[:, 0:S]
                sin2t = cst[:, S:2 * S]

                wgs = [wp.tile([P, NG * WCOLS], BF16, name=f"wg{g}")
                       for g in range(GPT)]
                xgs = [xp.tile([P, NG * TW], BF16, name=f"xg{i}")
                       for i in range(2 * GPT)]

                psq = [big0[:, 0:TW], big0[:, TW:2 * TW],
                       big1[:, 0:TW], big1[:, TW:2 * TW]]
                psk = big2[:, 0:TW]
                psv = big2[:, TW:2 * TW]
                # v-transpose slots alternate PSUM banks; their evacuation
                # copies run on ACT
                pstA2 = [big3[:, 0:64], bigT[:, 64:128]]

                traw6 = [rp.tile([P, TW], BF16, name=f"traw{i}") for i in range(5)]
                rot2 = [rp.tile([P, TW], BF16, name=f"rot{i}") for i in range(2)]
                # last-tq rope staging lives in the persistent pool so its
                # rope ops can be deferred past the pool-close barrier
                trawL = [gp.tile([P, TW], BF16, name=f"trawL{i}")
                         for i in range(5)]
                rotL = [gp.tile([P, TW], BF16, name=f"rotL{i}")
                        for i in range(2)]
                # persistent: the last tq's v transposes are deferred past
                # the pool-close barrier into phase B
                vtmp2 = [gp.tile([P, TW], BF16, name=f"vtmp{i}") for i in range(2)]

                def rope_finish(traw, rot, dest):
                    """dest = traw*cos2 + roll64(traw)*sin2 (sign folded in
                    sin2), via cross-quadrant 64-wide DVE muls."""
                    nc.vector.tensor_mul(rot[0:64, :], traw[64:128, :],
                                         sin2t[64:128, tsl])
                    nc.vector.tensor_mul(rot[64:128, :], traw[0:64, :],
                                         sin2t[0:64, tsl])
                    nc.vector.tensor_mul(traw[:, :], traw[:, :], cos2t[:, tsl])
                    nc.vector.tensor_add(dest, traw[:, :], rot[:, :])

                def touch_ap(t_ap):
                    return nc.tensor.matmul(tps[0:1, 0:2], t_ap[:, 0:1],
                                            t_ap[:, 0:2], start=True, stop=True)

                def emit_v_transpose(vq, j, slots=None):
                    # deferred v evacuation: transpose vtmp block j of a
                    # finished tq into vt, interleaved into the next tq's
                    # matmul stream
                    vtmp = vtmp2[vq % 2]
                    pstA = (slots or pstA2)[j % 2]
                    nc.tensor.transpose(_bfv(pstA),
                                        vtmp[:, j * P:(j + 1) * P],
                                        ident[:, :])
                    tok = vq * TW + j * P
                    nc.scalar.copy(vt[:, tok:tok + P], _bfv(pstA))

                # first-group DMA pieces (chunk ranges): small first so PE
                # starts after ~1 chunk of transfer
                PIECES = [(0, 1), (1, 2), (2, 4), (4, 6), (6, 8)]
                PIECE_STARTS = {a for a, b in PIECES if a > 0}

                for tq in range(TQ):
                    tsl = slice(tq * TW, (tq + 1) * TW)
                    for g in range(GPT):
                        gi = tq * GPT + g
                        xg = xgs[gi % (2 * GPT)]
                        if tq == 0 and g == 0:
                            for pi, (a, b) in enumerate(PIECES):
                                nc.sync.dma_start(
                                    out=wgs[0][:, a * WCOLS: b * WCOLS]
                                    .rearrange("p (c n) -> p c n", n=WCOLS),
                                    in_=wcat[a:b]
                                    .rearrange("c p n -> p c n"))
                                nc.sync.dma_start(
                                    out=xg[:, a * TW: b * TW]
                                    .rearrange("p (c n) -> p c n", n=TW),
                                    in_=xTr[a:b, :, tsl]
                                    .rearrange("c p n -> p c n"))
                                if pi == 0:
                                    # ident rides behind the first piece
                                    nc.sync.dma_start(out=ident[:, :],
                                                      in_=identh[:, :])
                        else:
                            nc.sync.dma_start(
                                out=xg[:, :].rearrange("p (c n) -> p c n",
                                                       n=TW),
                                in_=xTr[g * NG:(g + 1) * NG, :, tsl]
                                .rearrange("c p n -> p c n"))
                        if tq == 0:
                            if g == 1:
                                # absorb the ident DMA wait once PE is warm
                                touch(ident)
                            if g > 0:
                                nc.sync.dma_start(
                                    out=wgs[g][:, :].rearrange(
                                        "p (c n) -> p c n", n=WCOLS),
                                    in_=wcat[g * NG:(g + 1) * NG]
                                    .rearrange("c p n -> p c n"))
                            if g == 2:
                                nc.sync.dma_start(out=cst[:, :],
                                                  in_=cossin[:, :])
                        elif tq == 1:
                            # stream wo and the mask tile behind phase A's
                            # own queue head
                            if g == 0:
                                nc.sync.dma_start(out=trimt[:, :],
                                                  in_=trim[:, :])
                            nc.sync.dma_start(
                                out=woh[:, g * DIM:(g + 1) * DIM],
                                in_=woTr[g, :, :])
                        tx = touch(xg)
                        tw = touch(wgs[g]) if tq == 0 else None
                        tx2 = tw2 = None
                        for j in range(NG):
                            if tq == 0 and g == 0 and j in PIECE_STARTS:
                                # per-piece probes for the split group
                                tx2 = touch_ap(xg[:, j * TW:])
                                tw2 = touch_ap(wgs[0][:, j * WCOLS:])
                            k = g * NG + j
                            st, sp = (k == 0), (k == KC - 1)
                            w0 = j * WCOLS
                            xsl = xg[:, j * TW:(j + 1) * TW]
                            mms = []
                            mms.append(nc.tensor.matmul(
                                psk, wgs[g][:, w0 + HPC * D: w0 + HPC * D + D],
                                xsl, start=st, stop=sp))
                            mms.append(nc.tensor.matmul(
                                psv, wgs[g][:, w0 + HPC * D + D: w0 + WCOLS],
                                xsl, start=st, stop=sp))
                            for h in range(HPC):
                                mms.append(nc.tensor.matmul(
                                    psq[h],
                                    wgs[g][:, w0 + h * D: w0 + (h + 1) * D],
                                    xsl, start=st, stop=sp))
                            deps = []
                            if j == 0:
                                deps = [tx] + ([tw] if tw is not None else [])
                            elif tx2 is not None:
                                deps = [tx2, tw2]
                                tx2 = tw2 = None
                            for dep in deps:
                                for mi in mms:
                                    add_dep_helper(mi.ins, dep.ins, sync=False,
                                                   reason="touch first")
                        if tq > 0:
                            emit_v_transpose(tq - 1, g)
                    # epilogue: free psum banks with copies split across DVE
                    # and ACT, earliest-finished bank first. In the last tq,
                    # psq0/psq1 (the first phase-B score slots) go first.
                    tw6 = trawL if tq == TQ - 1 else traw6
                    if tq == TQ - 1:
                        # ACT frees the first two score slots then is clear
                        # for phase-B exps; DVE takes the rest (its phase-B
                        # work starts later)
                        nc.scalar.copy(tw6[0][:, :], psq[0])
                        nc.scalar.copy(tw6[2][:, :], psq[2])
                        nc.vector.tensor_copy(tw6[1][:, :], psq[1])
                        nc.vector.tensor_copy(tw6[3][:, :], psq[3])
                        nc.vector.tensor_copy(vtmp2[tq % 2][:, :], psv)
                        nc.vector.tensor_copy(tw6[4][:, :], psk)
                    else:
                        nc.vector.tensor_copy(tw6[4][:, :], psk)
                        nc.scalar.copy(vtmp2[tq % 2][:, :], psv)
                        nc.vector.tensor_copy(tw6[0][:, :], psq[0])
                        nc.scalar.copy(tw6[1][:, :], psq[1])
                        nc.vector.tensor_copy(tw6[2][:, :], psq[2])
                        nc.scalar.copy(tw6[3][:, :], psq[3])
                    if tq < TQ - 1:
                        for h in range(HPC):
                            rope_finish(traw6[h], rot2[h % 2],
                                        qTt[:, h * S + tq * TW: h * S + (tq + 1) * TW])
                        rope_finish(traw6[4], rot2[0], kTt[:, tsl])

                # Engine fences: Pool observes the last tq's DMA ticks
                # before the pools close; any leftover release waits on
                # other engines spill onto NoOps via _prune_waits
                fence = gp.tile([1, 40], F32, name="fence")
                last_slots = [((TQ - 1) * GPT + g) % (2 * GPT)
                              for g in range(GPT)]
                for i in last_slots:
                    nc.gpsimd.tensor_copy(fence[0:1, i:i + 1],
                                          xgs[i][0:1, 0:1])

            # ---------------- Phase B: attention + output projection ---
            with tc.tile_pool(name="attP", bufs=1) as attp:
                with tc.tile_pool(name="attw", bufs=1) as ap_:
                    # Pool barriers: a DVE memset of a tiny tile per pool
                    # absorbs the multi-engine pool-release waits; a PE
                    # fence + touch reduces the PE side to one self-wait.
                    barps = []
                    for pool, pnm in ((attp, "bA"), (ap_, "bw")):
                        bt = pool.tile([P, 2], F32, name=f"bar_{pnm}")
                        ms = nc.gpsimd.memset(bt[:, :], 0.0)
                        barps.append((bt, ms))
                    endA = touch(vt[:, S - TW - 2:S - TW])
                    for _, ms in barps:
                        add_dep_helper(endA.ins, ms.ins, sync=True,
                                       reason="fence observes pool barriers")
                    bars = []
                    for bt, _ in barps:
                        b = touch(bt)
                        add_dep_helper(b.ins, endA.ins, sync=False,
                                       reason="fence before pool barrier")
                        bars.append(b)
                    bar = bars[-1]
                    # ACT barrier: first ACT instruction touching attw tiles
                    # carries the ACT-visible release waits
                    nc.scalar.copy(barps[1][0][:, 0:1], barps[0][0][:, 0:1])

                    attT = attp.tile([P, HPC * S], BF16, name="attTt")

                    # E^T strips: strip sb[kb] holds exp(S^T) row kb at its
                    # natural q columns; 3-buffer rotation across (h, seq)
                    strips = [[ap_.tile([P, LMAX], BF16, name=f"strip{b}_{kb}")
                               for kb in range(max(seq_tiles))]
                              for b in range(3)]
                    ots1 = [ap_.tile([P, 2048], BF16, name=f"ots1_{i}")
                            for i in range(4)]
                    # softmax denominator machinery (2-deep rotation)
                    rr_sb = [ap_.tile([P, 8], F32, name=f"rrsb{i}")
                             for i in range(2)]
                    rr_sbb = [ap_.tile([P, 8], BF16, name=f"rrsbb{i}")
                              for i in range(2)]
                    rrT_row = [ap_.tile([1, LMAX], BF16, name=f"rrT{i}")
                               for i in range(2)]
                    rr_bc = [ap_.tile([P, LMAX], BF16, name=f"rrbc{i}")
                             for i in range(2)]
                    ones_t = ap_.tile([P, 2], BF16, name="ones")
                    mso = nc.vector.memset(ones_t[:, :], 1.0)
                    ones_touch = touch_ap(ones_t)
                    add_dep_helper(ones_touch.ins, mso.ins, sync=True,
                                   reason="ones ready")

                    # score/wo psum: five one-bank slots; rows wider than
                    # 512 use two-bank views aliasing slots (Tile
                    # serializes those)
                    ps_s5 = [big0[:, 0:512], big0[:, 512:1024],
                             big1[:, 0:512], big1[:, 512:1024],
                             big3[:, 0:512]]
                    ps_sbig = [big0[:, 0:LMAX], big1[:, 0:LMAX]]
                    se_ps2 = [bigT[:, 0:8], bigT[:, 8:16]]
                    # row-0 psum slots for the per-q-tile [128,1]->[1,128]
                    # rr transposes (walrus: partition offsets must be 0 or
                    # a quadrant start, so rr lands in row 0 contiguously)
                    rr_slots = ([_bfv(bigT[0:1, 128 + 64 * qt:192 + 64 * qt])
                                 for qt in range(4)] +
                                [_bfv(big2[0:1, 768 + 64 * j:832 + 64 * j])
                                 for j in range(2)])

                    woh_t = touch(woh)
                    add_dep_helper(woh_t.ins, bar.ins, sync=False,
                                   reason="after barrier")

                    # Deferred boundary work, spread one task per early
                    # attention head so the DVE rope burst doesn't queue
                    # ahead of the first sequences' softmax chains:
                    #  - last-tq v transposes (PE; slots in big2's tail so
                    #    they can't collide with the big3 score slot)
                    #  - last-tq RoPE into the L tiles (DVE; only gates the
                    #    last-quarter sequence, processed third)
                    vslots = [big2[:, 896:960], big2[:, 960:1024]]
                    tsl = slice((TQ - 1) * TW, TQ * TW)
                    boundary_tasks = []
                    boundary_tasks.append(
                        lambda: [emit_v_transpose(TQ - 1, j, slots=vslots)
                                 for j in range(TW // P)])
                    for h in range(HPC):
                        boundary_tasks.append(
                            lambda h=h: rope_finish(
                                trawL[h], rotL[h % 2],
                                qTtL[:, h * TW:(h + 1) * TW]))
                    boundary_tasks.append(
                        lambda: rope_finish(trawL[4], rotL[0], kTtL[:, :]))

                    def q_ap(h, tok, w):
                        if tok >= LQ0:
                            o = tok - LQ0
                            return qTtL[:, h * TW + o: h * TW + o + w]
                        return qTt[:, h * S + tok: h * S + tok + w]

                    def k_ap(tok, w):
                        if tok >= LQ0:
                            return kTtL[:, tok - LQ0: tok - LQ0 + w]
                        return kTt[:, tok - 0: tok + w]

                    # rotation starts on big3: the only phase-A traffic there
                    # is the tiny v-transpose slot, so it frees first
                    state = {"it": 0, "itb": 0, "ctile": 0, "nwo": 0}
                    pending = []   # deferred wo quarter-tile emitters

                    half_mode = {}

                    def emit_wo_quarter(mt, half, qi, ot):
                        # 4 head-chunk matmuls for one 512-col slice of the
                        # output row-tile, staged into ot; the 4th quarter
                        # fires the half-row DMA (or, during the final
                        # flush, each quarter DMAs itself).
                        if qi == 0:
                            half_mode[(mt, half)] = bool(state.get("flush"))
                        fine = half_mode[(mt, half)]
                        nch = 4 * half + qi
                        ps = ps_s5[state["it"] % 5]
                        state["it"] += 1
                        first = state["nwo"] == 0
                        state["nwo"] += 1
                        for f in range(HPC):
                            mi = nc.tensor.matmul(
                                ps,
                                attT[:, f * S + mt * P:
                                     f * S + (mt + 1) * P],
                                woh[:, f * DIM + nch * 512:
                                    f * DIM + (nch + 1) * 512],
                                start=(f == 0), stop=(f == HPC - 1))
                            if first:
                                add_dep_helper(
                                    mi.ins, woh_t.ins, sync=False,
                                    reason="woh touch first")
                        osl = ot[:, qi * 512:(qi + 1) * 512]
                        if mt % 2 == 0:
                            nc.scalar.copy(osl, ps)
                        else:
                            nc.vector.tensor_copy(osl, ps)
                        if fine:
                            nc.sync.dma_start(
                                out=out_d[mt * P:(mt + 1) * P,
                                          nch * 512:(nch + 1) * 512],
                                in_=ot[:, qi * 512:(qi + 1) * 512])
                        elif qi == 3:
                            nc.sync.dma_start(
                                out=out_d[mt * P:(mt + 1) * P,
                                          half * 2048:(half + 1) * 2048],
                                in_=ot[:, :])

                    def queue_wo(mt, half):
                        ot = ots1[state["ctile"] % 4]
                        state["ctile"] += 1
                        for qi in range(4):
                            pending.append(
                                lambda mt=mt, half=half, qi=qi, ot=ot:
                                emit_wo_quarter(mt, half, qi, ot))

                    def drain(n):
                        for _ in range(min(n, len(pending))):
                            pending.pop(0)()

                    # smallest sequence first (no wo backlog to hide), then
                    # ascending — except the largest runs second-to-last so
                    # the final sequence still has wo backlog to fill its
                    # attention gaps
                    s0s = []
                    acc = 0
                    for T in seq_tiles:
                        s0s.append(acc)
                        acc += T
                    order = sorted(range(len(seq_tiles)),
                                   key=lambda s: (seq_tiles[s], s))
                    if len(order) >= 3:
                        order[-1], order[-2] = order[-2], order[-1]

                    first_attn = [True]

                    def rows_gen(h, sb, se_ps, kb0, T, W):
                        """Generator: emit score row kb + exp + (kb-1)'s se
                        group per step. Yields after each row so two heads
                        can interleave."""
                        for kb in range(T):
                            Wk = W - kb * P
                            t0 = kb0 + kb * P  # diag global token
                            if Wk <= 512:
                                ps = ps_s5[state["it"] % 5]
                                state["it"] += 1
                            else:
                                ps = ps_sbig[state["itb"] % 2]
                                state["itb"] += 1
                            ksl = k_ap(t0, P)
                            # diagonal block + causal mask as one
                            # accumulation group
                            m1 = nc.tensor.matmul(
                                ps[:, 0:P], ksl, q_ap(h, t0, P),
                                start=True, stop=False)
                            if first_attn[0]:
                                first_attn[0] = False
                                add_dep_helper(m1.ins, bar.ins, sync=False,
                                               reason="bar first")
                            tmi = nc.tensor.matmul(
                                ps[:, 0:P], ident[:, :], trimt[:, :],
                                start=False, stop=True)
                            off = P
                            while off < Wk:
                                # chunks must not cross the 512-col psum
                                # bank boundary
                                w = min(512 - (off % 512), Wk - off)
                                nc.tensor.matmul(
                                    ps[:, off:off + w], ksl,
                                    q_ap(h, t0 + off, w),
                                    start=True, stop=True)
                                off += w
                            # exp row kb -> strip (no max subtraction:
                            # |scale*s| < ~25 keeps fp32 exp finite and
                            # softmax is shift-invariant)
                            ei = nc.scalar.activation(
                                sb[kb][:, kb * P: kb * P + Wk],
                                ps[:, 0:Wk], ACT.Exp, scale=SCALE)
                            add_dep_helper(ei.ins, tmi.ins, sync=True,
                                           reason="exp after mask")
                            # denominator group for q-tile kb-1 (its exp
                            # rows are all issued)
                            if kb >= 1:
                                qt = kb - 1
                                for kb2 in range(qt + 1):
                                    nc.tensor.matmul(
                                        se_ps[:, qt:qt + 1],
                                        sb[kb2][:, qt * P:(qt + 1) * P],
                                        ones_t[:, 0:1],
                                        start=(kb2 == 0), stop=(kb2 == qt))
                            if len(pending) > 4:
                                drain(1)
                            yield
                        qt = T - 1
                        for kb2 in range(qt + 1):
                            nc.tensor.matmul(
                                se_ps[:, qt:qt + 1],
                                sb[kb2][:, qt * P:(qt + 1) * P],
                                ones_t[:, 0:1],
                                start=(kb2 == 0), stop=(kb2 == qt))

                    def finish_head(h, sb, r, se_ps, kb0, T, W, pvb,
                                    slots):
                        """recip/cast, PV chunks (+rr transposes after the
                        first chunk), rr row staging, broadcast DMA.
                        pvb: column base of the PV accumulator in big2."""
                        nc.vector.reciprocal(rr_sb[r][:, 0:T],
                                             se_ps[:, 0:T])
                        nc.vector.tensor_copy(rr_sbb[r][:, 0:T],
                                              rr_sb[r][:, 0:T])
                        did_tr = False
                        for c0 in range(0, W, 512):
                            c1 = min(c0 + 512, W)
                            nkb = c1 // P
                            for kb in range(nkb):
                                r0 = max(c0, kb * P)
                                kt0 = kb0 + kb * P
                                nc.tensor.matmul(
                                    big2[:, pvb + r0:pvb + c1],
                                    vt[:, kt0:kt0 + P],
                                    sb[kb][:, r0:c1],
                                    start=(kb == 0), stop=(kb == nkb - 1))
                            if not did_tr:
                                did_tr = True
                                for qt in range(T):
                                    nc.tensor.transpose(
                                        slots[qt],
                                        rr_sbb[r][:, qt:qt + 1],
                                        ident[:, :])
                                drain(1)
                        # stage the rr row contiguously in SBUF, then
                        # replicate it across partitions with a broadcast
                        # DMA (stride-0 source dim)
                        Tb = min(T, 4)
                        nc.vector.tensor_copy(
                            rrT_row[r][0:1, 0:Tb * P],
                            _bfv(bigT[0:1, 128:128 + 64 * Tb]))
                        if T > 4:
                            nc.vector.tensor_copy(
                                rrT_row[r][0:1, 512:T * P],
                                _bfv(big2[0:1, 768:768 + 64 * (T - 4)]))
                        sl = rrT_row[r][0:1, 0:W]
                        src_bc = bass.AP(
                            sl.tensor, sl.offset,
                            [[sl.ap[0][0], 1], [0, P]]
                            + [list(d) for d in sl.ap[1:]])
                        nc.sync.dma_start(out=rr_bc[r][:, 0:W], in_=src_bc)

                    def evac_head(h, r, kb0, W, pvb):
                        # normalize while evacuating PV psum
                        nc.vector.tensor_mul(
                            attT[:, h * S + kb0: h * S + kb0 + W],
                            big2[:, pvb:pvb + W], rr_bc[r][:, 0:W])

                    hs = 0
                    for si in order:
                        T = seq_tiles[si]
                        s0 = s0s[si]
                        kb0 = s0 * P  # token offset of sequence start
                        W = T * P
                        if T <= 4:
                            # head pairs: B's PV lands in big2's second
                            # half, so each head's rr-broadcast latency
                            # hides under its partner's PV + evac
                            for hA in range(0, HPC, 2):
                                hB = hA + 1
                                sbA = strips[hs % 3]
                                sbB = strips[(hs + 1) % 3]
                                hs += 2
                                # rope/v-transpose boundary work lands at
                                # pair starts (rows are PE-heavy, DVE is
                                # idle); never on the very first pair, whose
                                # rows must not queue behind the v
                                # transposes' psv wait on PE
                                if hs > 2:
                                    for _ in range(2):
                                        if boundary_tasks:
                                            boundary_tasks.pop(0)()
                                gA = rows_gen(hA, sbA, se_ps2[0], kb0, T, W)
                                gB = rows_gen(hB, sbB, se_ps2[1], kb0, T, W)
                                for _ in range(T):
                                    next(gA)
                                    next(gB)
                                for g in (gA, gB):
                                    for _ in g:
                                        pass
                                finish_head(hA, sbA, 0, se_ps2[0], kb0, T,
                                            W, 0, rr_slots)
                                finish_head(hB, sbB, 1, se_ps2[1], kb0, T,
                                            W, 512, rr_slots)
                                evac_head(hA, 0, kb0, W, 0)
                                drain(2)
                                evac_head(hB, 1, kb0, W, 512)
                                drain(2)
                        else:
                            for h in range(HPC):
                                sb = strips[hs % 3]
                                r = hs % 2
                                hs += 1
                                for _ in rows_gen(h, sb, se_ps2[r], kb0,
                                                  T, W):
                                    pass
                                finish_head(h, sb, r, se_ps2[r], kb0, T,
                                            W, 0, rr_slots)
                                evac_head(h, r, kb0, W, 0)
                                drain(2)
                        # sequence complete for all heads: queue its output
                        # projection; emission dribbles into the next
                        # sequence's attention pipeline gaps
                        for mt in range(s0, s0 + T):
                            for half in range(2):
                                queue_wo(mt, half)
                    state["flush"] = True
                    while pending:
                        pending.pop(0)()

    _prune_waits(nc)
    return nc


def _prune_waits(nc):
    """Fit instructions into walrus's per-instruction sync-command budget
    (~2 commands: waits + updates; matmul LW and DMA take 1 wait).

    Recompute exact vector clocks over the emitted sem graph, drop waits
    implied by the proc's predecessor or by other kept waits' grants, and
    move any genuine overflow onto injected same-engine NoOps.
    """
    import concourse.mybir as _mybir

    f = nc.m.functions[0]

    CAP = {}
    SKIP = {"NoOp", "EventSemaphore", "AllEngineBarrier", "Halt"}
    DEFAULT_CAP = 1

    def join(a, b):
        for k, v in b.items():
            if a.get(k, -1) < v:
                a[k] = v
        return a

    sem_hist = {}    # sem id -> list of (cum_value_after, vec_of_updater)
    sem_cum = {}     # sem id -> cumulative value
    proc_vec = {}    # proc key -> vector of last completed inst on proc
    nop_n = [0]

    def proc_of(i):
        si = i.sync_info
        if i.opcode in ("DMACopy", "DMATranspose") and si and si.on_update:
            return ("sem", si.on_update[0].id)
        return ("eng", str(i.engine))

    def grant_vec(sem_id, value):
        for cum, vec in sem_hist.get(sem_id, []):
            if cum >= value:
                return vec
        return {}

    for bb in f.blocks:
        out_insts = []
        for i in bb.instructions:
            si = i.sync_info
            p = proc_of(i)
            base = dict(proc_vec.get(p, {}))
            myvec = dict(base)
            if si and si.on_wait:
                waits = list(si.on_wait)
                grants = []
                for w in waits:
                    if w.wait_mode == "sem-ge-imm" and w.wait_value is not None:
                        grants.append(grant_vec(w.id, w.wait_value))
                    else:
                        grants.append(None)  # unknown -> always keep
                # grants of ALL original waits flow into this proc's clock:
                # dropped waits are implied, spilled waits still execute on
                # a same-engine NoOp immediately before this instruction
                all_grants = list(grants)
                cap = CAP.get(i.opcode, DEFAULT_CAP)
                if i.opcode in SKIP:
                    cap = 99
                if len(waits) > cap:
                    keep = [True] * len(waits)
                    order = sorted(range(len(waits)),
                                   key=lambda k: -(waits[k].wait_value or 0))
                    for k in order:
                        if sum(keep) <= cap:
                            break
                        if grants[k] is None:
                            continue
                        w = waits[k]
                        cov = dict(base)
                        for j2 in range(len(waits)):
                            if j2 != k and keep[j2] and grants[j2] is not None:
                                join(cov, grants[j2])
                        if cov.get(("sem", w.id), -1) >= (w.wait_value or 0):
                            keep[k] = False
                    kept = [w for k2, w in enumerate(waits) if keep[k2]]
                    if len(kept) > cap:
                        # move overflow onto same-engine NoOps (<=2 each)
                        if i.opcode in ("DMACopy", "DMATranspose"):
                            import sys
                            print(f"WAITPRUNE: cannot nop-split DMA {i.name}: "
                                  f"{[(w.ant_name, w.wait_value) for w in kept]}",
                                  file=sys.stderr)
                        else:
                            overflow = kept[cap:]
                            kept = kept[:cap]
                            for c0 in range(0, len(overflow), 1):
                                chunk = overflow[c0:c0 + 1]
                                nop_n[0] += 1
                                nop = _mybir.InstNoOp(
                                    name=f"WPNOP-{nop_n[0]}",
                                    engine=i.engine,
                                    ins=[], outs=[],
                                    sync_info=_mybir.SyncInfo(
                                        on_wait=chunk, on_update=[]),
                                )
                                out_insts.append(nop)
                    si.on_wait = kept
                for g in all_grants:
                    if g is not None:
                        join(myvec, g)
            # complete this instruction on proc p
            myvec[p] = myvec.get(p, 0) + 1
            if si and si.on_update:
                for u in si.on_update:
                    cum = sem_cum.get(u.id, 0) + (u.update_value or 0)
                    sem_cum[u.id] = cum
                    myvec[("sem", u.id)] = cum
                    sem_hist.setdefault(u.id, []).append((cum, dict(myvec)))
            proc_vec[p] = myvec
            out_insts.append(i)
        bb.instructions = out_insts


_BUILD_CACHE = {}


def _get_nc(seq_tiles):
    key = tuple(seq_tiles)
    if key not in _BUILD_CACHE:
        _BUILD_CACHE[key] = _build(key)
    return _BUILD_CACHE[key]


def _prepare(x, freqs_cis, seqlens, wq, wk, wv, wo):
    BF = ml_dtypes.bfloat16
    x = np.asarray(x, dtype=np.float32)
    freqs_cis = np.asarray(freqs_cis, dtype=np.float32)
    sl = np.asarray(seqlens).astype(np.int64)
    wq = np.asarray(wq, dtype=np.float32)
    wk = np.asarray(wk, dtype=np.float32)
    wv = np.asarray(wv, dtype=np.float32)
    wo = np.asarray(wo, dtype=np.float32)

    assert int(sl.sum()) == S and all(int(v) % P == 0 for v in sl)
    seq_tiles = tuple(int(v) // P for v in sl)
    nc = _get_nc(seq_tiles)

    # host-side layout prep (shared across cores)
    xTr = np.ascontiguousarray(x.T.astype(BF)).reshape(KC, P, S)
    cos = np.ascontiguousarray(freqs_cis[:, :, 0].T)        # [64, S]
    sin = np.ascontiguousarray(freqs_cis[:, :, 1].T)
    cos2 = np.concatenate([cos, cos], 0)
    # sign folded in, keyed by INPUT row of the shifted mul
    sin2 = np.concatenate([sin, -sin], 0)
    cossin = np.ascontiguousarray(
        np.concatenate([cos2, sin2], 1).astype(BF))         # [128, 2S]
    # transposed-score causal mask: NEG where k (partition) > q (free)
    trimask = np.where(
        np.arange(P)[:, None] <= np.arange(P)[None, :], 0.0, NEG
    ).astype(BF)
    ident_np = np.eye(P, dtype=np.float32).astype(BF)
    perm = np.concatenate([np.arange(0, D, 2), np.arange(1, D, 2)])  # evens|odds

    in_maps = []
    for c in range(NCORES):
        qrows = (np.arange(HPC)[:, None] * D + c * HPC * D + perm[None, :]).ravel()
        krows = c * D + perm
        vrows = np.arange(c * D, (c + 1) * D)
        wcat = np.concatenate([
            wq[qrows].T, wk[krows].T, wv[vrows].T], axis=1)  # [DIM, 768]
        in_maps.append({
            "xTr": xTr,
            "wcat": np.ascontiguousarray(wcat.astype(BF)).reshape(KC, P, WCOLS),
            "woTr": np.ascontiguousarray(
                wo[:, c * HPC * D:(c + 1) * HPC * D].T.astype(BF)
            ).reshape(HPC, P, DIM),
            "cossin": cossin,
            "trim": trimask,
            "identh": ident_np,
        })

    return nc, in_maps


def kernel(x, freqs_cis, seqlens, wq, wk, wv, wo):
    global LAST_RESULTS
    nc, in_maps = _prepare(x, freqs_cis, seqlens, wq, wk, wv, wo)
    res = run_bass_kernel_spmd(
        nc, in_maps, core_ids=list(range(NCORES)),
        trace=bool(int(os.environ.get("KERNEL_TRACE", "0"))),
    )
    LAST_RESULTS = res
    acc = res.results[0]["out"].astype(np.float32)
    for r in res.results[1:]:
        acc = acc + r["out"].astype(np.float32)
    return acc
